# revision 16
# baseline (speedup 1.0000x reference)
"""GCN layer (message passing + linear + BatchNorm + ReLU + residual) on 8 TRN2 cores.

V2 strategy (graph/data parallel, per sharding hint):
  - Nodes (and their incident edges, grouped by dst tile) sharded across 8
    cores; dst tiles DEALT to (slot, core) by descending edge count so all
    cores see near-identical per-slot counts (SPMD shapes are cross-core max).
  - Node table relabeled by a greedy parity-balancing pass (discrepancy-style)
    so even/odd gather lists are balanced per tile; rows stored as bf16 only
    (no hi/lo pair) — full-pipeline rel err ~3e-3 vs the 2e-2 gate.
  - Feature rows gathered via SWDGE dma_gather (256B rows) from a replicated
    DRAM table; calls batched per (group, parity-view) up to the 1024-desc
    SWDGE ring cap (HW-bisected; resizing the ring crashes the device).
  - Per-tile edge lists packed at 16-granularity inside each group region;
    blocks shared by two tiles get one matmul per tile with complementary
    one-hot columns (dstloc = -1 masks foreign slots).
  - Segment-mean fused into the one-hot build: oh = (iota == dstloc) * recip
    via a single DVE tensor_scalar(is_equal, mult) — no separate recip
    multiply and no [128, shard] recip broadcast load.
  - Slots processed in QUADS of <=4 sharing one PSUM bank: one Act h1-copy,
    one 512-wide hi/lo W-matmul pair, one z-copy and one Square per quad
    (amortizes the Act engine's per-op PSUM/SBUF access latency 4x). Quads
    are flat runs over the slot sequence and may span gather groups, so the
    taper tail drains as a single quad.
  - Linear = W @ h1 with hi/lo split W (bf16 pair), h1 = bf16 copy of the
    PSUM aggregate (Act engine; GPSIMD cannot touch PSUM on HW).
  - BN batch stats: Sum(z) accumulated by the Act-engine PSUM->zbuf copy
    (accum_out, zbuf in bf16), Sum(z^2) by an Act Square pass; [128,2]
    AllReduce across cores; b omitted (training-mode BN is invariant to b).
  - Residual features streamed early (overlaps gathers); finalize writes out
    in ~1KB-wide chunks (pairs of quads) so the output DMA pipelines with
    the Act relu + DVE residual-add tail.

  Explored and rejected (cost model + HW experiments, see session notes):
  fp8/int8 message gathers (fp8 fails the 2e-2 gate at full rate and SWDGE
  descriptors have a 256B floor, so sub-256B rows do not reduce modeled DMA
  time; int8 is not a PE matmul dtype), >1024-desc SWDGE calls and ring
  resizing (crash the device), gpsimd ap_gather (0.6 efficiency makes it no
  cheaper than SWDGE desc-gen), indirect_dma_start bulk gathers (HW consumes
  one offset per partition, i.e. 128 rows/call max).
"""

import sys

for _p in ("/opt/trn_rl_repo", "/root/.axon_site/_ro/trn_rl_repo"):
    if _p not in sys.path:
        sys.path.insert(0, _p)

import numpy as np
from contextlib import ExitStack

import concourse.bass as bass
import concourse.bacc as bacc
import concourse.mybir as mybir
import concourse.tile as tile
from concourse.bass_utils import run_bass_kernel_spmd

try:
    from ml_dtypes import bfloat16 as np_bf16
except ImportError:  # jax always ships ml_dtypes
    import jax.numpy as _jnp

    np_bf16 = _jnp.bfloat16

F32 = mybir.dt.float32
BF16 = mybir.dt.bfloat16
I16 = mybir.dt.int16

P = 128          # partitions / tile node count / edge block size
D = 128          # feature dim
NCORES = 8
BN_EPS = 1e-5
GMAX = 1024      # SWDGE ring descriptor cap (HW-bisected: >1024 crashes, and
RING = 16384     # resizing dynamic_dma_scratch_size crashes even small calls)
GTILES = 7       # dst tiles per gather group
TAPER = (4, 2, 2, 1, 1)
MBUFS = 3


def _r16(x):
    return (int(x) + 15) // 16 * 16


# ---------------------------------------------------------------- host prep

def _parity_greedy(n, src, gtile, ntg, npad):
    """Choose a parity bit per node minimizing per-tile even/odd imbalance.

    Returns (par[n] in {0,1}, rank[n] = row within its parity class).
    """
    order = np.argsort(src, kind="stable")
    s_sorted = src[order]
    g_sorted = gtile[order]
    starts = np.searchsorted(s_sorted, np.arange(n + 1))
    outdeg = starts[1:] - starts[:-1]
    nord = np.argsort(-outdeg, kind="stable")

    imb = np.zeros(ntg, dtype=np.int64)
    par = np.zeros(n, dtype=np.int8)
    csize = [0, 0]
    half = npad // 2
    for v in nord:
        lo, hi = starts[v], starts[v + 1]
        if lo == hi:
            p = 0 if csize[0] <= csize[1] else 1
        else:
            tv = g_sorted[lo:hi]
            s = imb[tv].sum()
            if s > 0:
                p = 1
            elif s < 0:
                p = 0
            else:
                p = 0 if csize[0] <= csize[1] else 1
            if csize[p] >= half:          # class full: forced
                p = 1 - p
            np.add.at(imb, tv, 1 - 2 * p)
        par[v] = p
        csize[p] += 1
    rank = np.zeros(n, dtype=np.int64)
    for q in (0, 1):
        idx = np.flatnonzero(par == q)
        rank[idx] = np.arange(idx.shape[0])
    return par, rank


def host_prep(feature, W, gamma, beta, src, dst, ncores=NCORES):
    """Index-only graph preprocessing + data layout. Returns (cfg, in_maps)."""
    n, d = feature.shape
    assert d == D
    shard = -(-n // (ncores * P)) * P          # per-core node count, mult of 128
    npad = shard * ncores
    nt = shard // P                            # dst tiles per core
    ntg = ncores * nt

    src = np.asarray(src, dtype=np.int64)
    dst = np.asarray(dst, dtype=np.int64)

    deg = np.bincount(dst, minlength=npad).astype(np.float64)
    recip = np.where(deg > 0, 1.0 / np.maximum(deg, 1.0), 0.0).astype(np.float32)

    gtile = dst // P                           # global dst tile of each edge
    tile_counts = np.bincount(gtile, minlength=ntg)

    # deal tiles to (slot, core) by descending count: cross-core maxes shrink
    deal = np.argsort(-tile_counts, kind="stable").reshape(nt, ncores)

    # parity balancing + node relabel (table order only; dst untouched)
    par, rank = _parity_greedy(n, src, gtile, ntg, npad)

    # per-edge gather data
    e_par = par[src]                           # which view
    e_idx = rank[src].astype(np.int16)         # row within view
    e_slot = (dst % P).astype(np.float32)      # dst slot within its tile
    e_rec = recip[dst]                         # fold 1/deg into the one-hot

    # edges grouped by global tile
    eorder = np.argsort(gtile, kind="stable")
    tstarts = np.searchsorted(gtile[eorder], np.arange(ntg + 1))

    def tile_edges(g, q):
        ee = eorder[tstarts[g] : tstarts[g + 1]]
        ee = ee[e_par[ee] == q]
        return e_idx[ee], e_slot[ee], e_rec[ee]

    # per (slot, parity) padded length = r16(max over cores)
    cnt = np.zeros((nt, ncores, 2), dtype=np.int64)
    for s in range(nt):
        for c in range(ncores):
            g = deal[s][c]
            for q in (0, 1):
                ee = eorder[tstarts[g] : tstarts[g + 1]]
                cnt[s, c, q] = int((e_par[ee] == q).sum())
    pad_len = np.zeros((nt, 2), dtype=np.int64)
    for s in range(nt):
        for q in (0, 1):
            pad_len[s, q] = _r16(cnt[s, :, q].max())

    # groups of slots; per group: [ev region | od region], regions 128-block
    # aligned at region starts (ev region padded to full blocks). First and
    # last groups are small: compute starts sooner after the first gather,
    # and the serial compute tail after the final gather is short.
    taper = list(TAPER)
    sizes = [2]
    rem = nt - 2 - sum(taper)
    while rem > GTILES:
        sizes.append(GTILES)
        rem -= GTILES
    if rem:
        sizes.append(rem)
    sizes.extend(taper)
    groups = []
    pos0 = 0
    for sz in sizes:
        groups.append(list(range(pos0, pos0 + sz)))
        pos0 += sz
    assert pos0 == nt
    ginfo = []          # per group dict
    tile_prog = [[] for _ in range(nt)]   # per slot: list of (mb, col)
    ncols = 0
    icols = 0
    gbmax = 0
    for gi, g in enumerate(groups):
        # regions rounded to whole 128-blocks: every gathered slot is written
        # (pad indices are 0 -> valid row data; dstloc=-1 masks them), else
        # unwritten SBUF tail bytes (can be NaN) would flow into the matmul
        ev_len = -(-int(sum(pad_len[s, 0] for s in g)) // P) * P
        od_len = -(-int(sum(pad_len[s, 1] for s in g)) // P) * P
        ebg = ev_len // P
        obg = od_len // P
        nblk = ebg + obg
        gbmax = max(gbmax, ebg, obg)

        # per-view gather calls (split at the ring cap)
        gmax_g = GMAX
        def _calls(ln, blk0, ic0):
            out = []
            off = 0
            while ln - off > gmax_g:
                step = (gmax_g // P) * P
                out.append((ic0 + off // 16, step, blk0 + off // P))
                off += step
            if ln > off:
                out.append((ic0 + off // 16, ln - off, blk0 + off // P))
            return out

        ev_calls = _calls(ev_len, 0, icols)
        od_calls = _calls(od_len, 0, icols + ev_len // 16)

        # per-slot region offsets and (view, block, col) program; blocks are
        # view-relative (ev and od live in separate msgs tiles so compute on
        # one view's blocks never waits for the other view's gather)
        col_meta = []   # (slot, q, region_pos0, blk)  for building dl/re later
        for q in (0, 1):
            pos = 0
            for s in g:
                ln = int(pad_len[s, q])
                if ln == 0:
                    continue
                b0 = pos // P
                b1 = -(-(pos + ln) // P)
                for b in range(b0, b1):
                    tile_prog[s].append((q, b, ncols))
                    col_meta.append((s, q, pos, b))
                    ncols += 1
                pos += ln
        ginfo.append(
            {
                "slots": g,
                "ev_len": ev_len,
                "od_len": od_len,
                "ebg": ebg,
                "obg": obg,
                "nblk": nblk,
                "ev_calls": ev_calls,
                "od_calls": od_calls,
                "icol0": icols,
                "col_meta": col_meta,
            }
        )
        icols += ev_len // 16 + od_len // 16

    total_desc = sum(gg["ev_len"] + gg["od_len"] for gg in ginfo)

    # ---- per-core arrays ----
    xpad = np.zeros((npad, D), dtype=np.float32)
    xpad[:n] = np.asarray(feature, dtype=np.float32)
    fbf = xpad[:n].astype(np_bf16)
    xsbf = np.zeros((npad // 2, 2, D), dtype=np_bf16)
    for q in (0, 1):
        idxq = np.flatnonzero(par == q)
        xsbf[rank[idxq], q, :] = fbf[idxq]

    wt = np.ascontiguousarray(np.asarray(W, dtype=np.float32).T)
    wthi = wt.astype(np_bf16)
    wtlo = (wt - wthi.astype(np.float32)).astype(np_bf16)
    wtcat = np.concatenate([wthi, wtlo], axis=1)             # [128, 256] bf16

    iotab = np.ascontiguousarray(
        np.broadcast_to(np.arange(P, dtype=np.float32), (P, P)).astype(np_bf16)
    )
    gb = np.stack(
        [np.asarray(gamma, np.float32), np.asarray(beta, np.float32)], axis=1
    )

    col_desc = []  # (gi, s, q, pos0, b) in sequential col-id order
    for gi, gg in enumerate(ginfo):
        for s, q, pos0, b in gg["col_meta"]:
            col_desc.append((gi, s, q, pos0, b))
    assert len(col_desc) == ncols

    in_maps = [None] * ncores
    rows_of_core = []
    for c in range(ncores):
        rows = np.concatenate(
            [np.arange(deal[s][c] * P, deal[s][c] * P + P) for s in range(nt)]
        )
        rows_of_core.append(rows)
        idx_cols = np.zeros((16, icols), dtype=np.int16)
        dl = np.full((P, ncols), -1.0, dtype=np.float32)
        re = np.zeros((P, ncols), dtype=np.float32)

        for gg in ginfo:
            for q in (0, 1):
                ic = gg["icol0"] + (gg["ev_len"] // 16 if q == 1 else 0)
                for s in gg["slots"]:
                    ln = int(pad_len[s, q])
                    if ln == 0:
                        continue
                    ii, _, _ = tile_edges(deal[s][c], q)
                    vals = np.zeros(ln, dtype=np.int16)
                    vals[: ii.shape[0]] = ii
                    idx_cols[:, ic : ic + ln // 16] = vals.reshape(ln // 16, 16).T
                    ic += ln // 16

        # per-column dstloc/recip: column (s, q, pos0, b): block b covers
        # region positions [b*P, (b+1)*P); the slot's edges occupy
        # [pos0, pos0+cnt(c,s,q)) with dl = slot, re = recip
        for colid, (gi, s, q, pos0, b) in enumerate(col_desc):
            _, ss, rr = tile_edges(deal[s][c], q)
            m = ss.shape[0]
            lo = b * P
            dlcol = np.full(P, -1.0, dtype=np.float32)
            recol = np.zeros(P, dtype=np.float32)
            # positions of this slot's real edges inside this block
            p0 = max(pos0, lo)
            p1 = min(pos0 + m, lo + P)
            if p1 > p0:
                dlcol[p0 - lo : p1 - lo] = ss[p0 - pos0 : p1 - pos0]
                recol[p0 - lo : p1 - lo] = rr[p0 - pos0 : p1 - pos0]
            dl[:, colid] = dlcol
            re[:, colid] = recol

        xt = np.ascontiguousarray(xpad[rows].T.astype(np_bf16))  # [128, shard] bf16

        in_maps[c] = {
            "xsbf": np.ascontiguousarray(xsbf),
            "xt": xt,
            "idx": np.ascontiguousarray(np.tile(idx_cols, (8, 1))),
            "dl": np.ascontiguousarray(dl),
            "re": np.ascontiguousarray(re),
            "wt": np.ascontiguousarray(wtcat),
            "iotab": iotab,
            "gb": np.ascontiguousarray(gb),
        }

    cfg = {
        "n": n,
        "npad": npad,
        "shard": shard,
        "nt": nt,
        "ncores": ncores,
        "icols": icols,
        "ncols": ncols,
        "gbmax": gbmax,
        "ginfo": ginfo,
        "tile_prog": tile_prog,
        "total_desc": total_desc,
        "rows_of_core": rows_of_core,
    }
    return cfg, in_maps


# ---------------------------------------------------------------- device program

def build_program(cfg, skip_collective=False):
    ncores = cfg["ncores"]
    shard, nt = cfg["shard"], cfg["nt"]
    npad = cfg["npad"]
    icols, ncols = cfg["icols"], cfg["ncols"]
    gbmax = cfg["gbmax"]
    ginfo = cfg["ginfo"]
    tile_prog = cfg["tile_prog"]
    inv_n = 1.0 / cfg["n"]

    nc = bacc.Bacc("TRN2", target_bir_lowering=False, debug=False,
                   num_devices=ncores, dynamic_dma_scratch_size=RING)

    xsbf = nc.declare_dram_parameter("xsbf", [npad // 2, 2, D], BF16, False)
    xt_d = nc.declare_dram_parameter("xt", [P, shard], BF16, False)
    idx_d = nc.declare_dram_parameter("idx", [P, icols], I16, False)
    dl_d = nc.declare_dram_parameter("dl", [P, ncols], F32, False)
    re_d = nc.declare_dram_parameter("re", [P, ncols], F32, False)
    wt_d = nc.declare_dram_parameter("wt", [P, 2 * D], BF16, False)
    io_d = nc.declare_dram_parameter("iotab", [P, P], BF16, False)
    gb_d = nc.declare_dram_parameter("gb", [P, 2], F32, False)
    out_d = nc.declare_dram_parameter("outt", [P, shard], BF16, True)

    AL = mybir.AluOpType
    AF = mybir.ActivationFunctionType

    with ExitStack() as ctx:
        tc = ctx.enter_context(tile.TileContext(nc))
        const = ctx.enter_context(tc.tile_pool(name="const", bufs=1))
        mpool = ctx.enter_context(tc.tile_pool(name="msgs", bufs=MBUFS))
        ohpool = ctx.enter_context(tc.tile_pool(name="oh", bufs=20))
        hpool = ctx.enter_context(tc.tile_pool(name="h1", bufs=3))
        scpool = ctx.enter_context(tc.tile_pool(name="scratch", bufs=2))
        fpool = ctx.enter_context(tc.tile_pool(name="fin", bufs=4))
        stat = ctx.enter_context(tc.tile_pool(name="stat", bufs=1))
        apool = ctx.enter_context(tc.tile_pool(name="aggps", bufs=2, space="PSUM"))
        zpool = ctx.enter_context(tc.tile_pool(name="zps", bufs=2, space="PSUM"))
        dram = ctx.enter_context(tc.tile_pool(name="dram", bufs=2, space="DRAM"))

        ic_split = ginfo[0]["icol0"] + (ginfo[0]["ev_len"] + ginfo[0]["od_len"]) // 16
        idx_s0 = const.tile([P, ic_split], I16)
        idx_s1 = const.tile([P, icols - ic_split], I16)
        dl_s = const.tile([P, ncols], F32)
        re_s = const.tile([P, ncols], F32)
        wt_s = const.tile([P, 2 * D], BF16)
        io_s = const.tile([P, P], BF16)
        gb_s = const.tile([P, 2], F32)
        # slots processed in QUADS of <=4 (one PSUM bank): one Act h1-copy,
        # one wide z-matmul pair, one z-copy and one Square per quad instead
        # of per slot -- the Act engine's per-op SBUF/PSUM access latency is
        # amortized 4x. Quads are flat runs over the slot sequence and MAY
        # span gather groups (dep tracking orders them after the gathers they
        # read; the taper tail then drains as one quad, not 3-4 tiny ones).
        quads = []                     # [slots]
        i_ = 0
        while i_ < nt:
            take = min(4, nt - i_)
            if nt - i_ == 5:
                take = 3               # split 5 as 3+2, not 4+1
            quads.append(list(range(i_, i_ + take)))
            i_ += take
        nquads = len(quads)
        group_of_slot = {}
        for gi_, gg_ in enumerate(ginfo):
            for s_ in gg_["slots"]:
                group_of_slot[s_] = gi_
        # a quad is computed once the group of its LAST slot has gathered
        quads_done_in_group = {gi_: [] for gi_ in range(len(ginfo))}
        for qi_, qs_ in enumerate(quads):
            quads_done_in_group[group_of_slot[qs_[-1]]].append(qi_)
        # finalize chunks = pairs of quads (~8 tiles / 1 KB-wide Act ops);
        # per-chunk zbuf tiles keep the finalize pipeline from serializing
        # against one big buffer via tile-granular deps
        zchunks = [list(range(k, min(k + 2, nquads))) for k in range(0, nquads, 2)]
        zc_of_quad = {}
        for zc_, ql_ in enumerate(zchunks):
            off_ = 0
            for qi_ in ql_:
                zc_of_quad[qi_] = (zc_, off_)
                off_ += len(quads[qi_]) * P
        zwidth = [sum(len(quads[qi_]) for qi_ in ql_) * P for ql_ in zchunks]
        zbufs = [const.tile([P, zwidth[zc_]], BF16, name=f"zbuf{zc_}")
                 for zc_ in range(len(zchunks))]
        xtbuf = const.tile([P, shard], BF16)
        sacc = const.tile([P, nquads], F32)
        qacc = const.tile([P, nquads], F32)

        # first group's idx columns first (its gathers start ~1.5us earlier),
        # then the rest; xt deferred to the compute tail after the last gather
        # so it never delays gather descriptors
        nc.sync.dma_start(idx_s0[:], idx_d[:, 0:ic_split])
        nc.sync.dma_start(idx_s1[:], idx_d[:, ic_split:icols])
        nc.sync.dma_start(io_s[:], io_d[:])
        nc.sync.dma_start(dl_s[:], dl_d[:])
        nc.sync.dma_start(re_s[:], re_d[:])
        nc.sync.dma_start(wt_s[:], wt_d[:])
        nc.sync.dma_start(gb_s[:], gb_d[:])

        # dummy Sqrt: selects the act-function set covering Copy, Square,
        # Relu AND Sqrt ("sqrt_and_others"), so no mid-stream table reload
        # stalls the Act engine before the BN scalar chain
        epsb = stat.tile([P, 1], F32)
        nc.vector.memset(epsb[:], float(BN_EPS))
        warm = stat.tile([P, 1], F32)
        nc.scalar.activation(warm[:], epsb[:], AF.Sqrt)

        ev_view = xsbf[:, 0, :]   # [npad//2, D] stride 2*D
        od_view = xsbf[:, 1, :]

        ngroups = len(ginfo)
        group_msgs = {}
        for gi, gg in enumerate(ginfo):
            msgse = mpool.tile([P, gbmax, D], BF16, tag="msgse")
            msgso = mpool.tile([P, gbmax, D], BF16, tag="msgso")
            group_msgs[gi] = (msgse, msgso)
            for view, mt, calls in ((ev_view, msgse, gg["ev_calls"]),
                                    (od_view, msgso, gg["od_calls"])):
                for ic0, ln, blk0 in calls:
                    if ic0 < ic_split:
                        idx_ap = idx_s0[:, ic0 : ic0 + ln // 16]
                    else:
                        idx_ap = idx_s1[:, ic0 - ic_split : ic0 - ic_split + ln // 16]
                    nc.gpsimd.dma_gather(
                        mt[:, blk0 : blk0 + (-(-ln // P)), :],
                        view,
                        idx_ap,
                        ln,
                        ln,
                        elem_size=D,
                        elem_step=2 * D,
                    )

            for qi in quads_done_in_group[gi]:
                qs = quads[qi]
                w = len(qs) * P
                aggq = apool.tile([P, 4 * P], F32, tag="agg")
                for j, s in enumerate(qs):
                    prog = tile_prog[s]
                    msgs_q = group_msgs[group_of_slot[s]]
                    nk = len(prog)
                    for k, (q, mb, col) in enumerate(prog):
                        oh = ohpool.tile([P, P], BF16, tag="oh")
                        nc.vector.tensor_scalar(
                            oh[:], io_s[:],
                            dl_s[:, col : col + 1], re_s[:, col : col + 1],
                            AL.is_equal, AL.mult,
                        )
                        nc.tensor.matmul(
                            aggq[:, j * P : (j + 1) * P],
                            msgs_q[q][:, mb, :], oh[:],
                            start=(k == 0), stop=(k == nk - 1),
                        )

                h1q = hpool.tile([P, 4 * P], BF16, tag="h1")
                nc.scalar.activation(h1q[:, 0:w], aggq[:, 0:w], AF.Copy)

                zq = zpool.tile([P, 4 * P], F32, tag="zp")
                nc.tensor.matmul(zq[:, 0:w], wt_s[:, 0:D], h1q[:, 0:w],
                                 start=True, stop=False)
                nc.tensor.matmul(zq[:, 0:w], wt_s[:, D : 2 * D], h1q[:, 0:w],
                                 start=False, stop=True)

                # PSUM -> zbuf copy with Sum(z) accumulation; Square pass for
                # Sum(z^2). Both on the Act engine (PE/DVE stay free).
                zc, zoff = zc_of_quad[qi]
                nc.scalar.activation(
                    zbufs[zc][:, zoff : zoff + w], zq[:, 0:w],
                    AF.Copy, accum_out=sacc[:, qi : qi + 1],
                )
                sq = scpool.tile([P, 4 * P], BF16, tag="sq")
                nc.scalar.activation(
                    sq[:, 0:w], zq[:, 0:w], AF.Square,
                    accum_out=qacc[:, qi : qi + 1]
                )

        # residual stream issued on the Pool SWDGE queue: its descriptor gen
        # queues behind the final gather's, so the transfer lands in the
        # post-gather tail instead of ahead of the last gathers (the idle SP
        # engine would otherwise request the DMA unit early and push the
        # gather stream's end out by the full xt transfer time)
        nc.gpsimd.dma_start(xtbuf[:], xt_d[:])

        # ---- BatchNorm statistics (global over all cores) ----
        ssum = stat.tile([P, 2], F32)
        nc.vector.tensor_reduce(
            ssum[:, 0:1], sacc[:], axis=mybir.AxisListType.X, op=AL.add
        )
        nc.vector.tensor_reduce(
            ssum[:, 1:2], qacc[:], axis=mybir.AxisListType.X, op=AL.add
        )

        tot = stat.tile([P, 2], F32)
        if skip_collective:
            tot = ssum
        else:
            cin = dram.tile([P, 2], F32)
            cout = dram.tile([P, 2], F32)
            nc.gpsimd.dma_start(cin[:], ssum[:])
            nc.gpsimd.collective_compute(
                "AllReduce",
                AL.add,
                replica_groups=[list(range(ncores))],
                ins=[cin.opt()],
                outs=[cout.opt()],
            )
            nc.gpsimd.dma_start(tot[:], cout[:])

        # scale = gamma / sqrt(var+eps); shift = beta - mu*scale
        m2 = stat.tile([P, 2], F32)
        nc.vector.tensor_scalar(m2[:], tot[:], inv_n, None, AL.mult)
        mu = m2[:, 0:1]
        var = stat.tile([P, 1], F32)
        nc.vector.tensor_mul(var[:], m2[:, 0:1], m2[:, 0:1])
        nc.vector.tensor_sub(var[:], m2[:, 1:2], var[:])
        sd = stat.tile([P, 1], F32)
        nc.scalar.activation(sd[:], var[:], AF.Sqrt, bias=epsb[:])
        inv = stat.tile([P, 1], F32)
        nc.vector.reciprocal(inv[:], sd[:])
        scale = stat.tile([P, 1], F32)
        nc.vector.tensor_mul(scale[:], gb_s[:, 0:1], inv[:])
        shift = stat.tile([P, 1], F32)
        nc.vector.tensor_mul(shift[:], m2[:, 0:1], scale[:])
        nc.vector.tensor_sub(shift[:], gb_s[:, 1:2], shift[:])

        # ---- finalize: out = x + relu(z*scale + shift); xt was preloaded,
        # output written per chunk so the store DMA overlaps the compute ----
        for zc, ql in enumerate(zchunks):
            w = zwidth[zc]
            c0 = quads[ql[0]][0] * P
            sl = slice(c0, c0 + w)
            tmp = fpool.tile([P, 8 * P], BF16, tag="fin")
            nc.scalar.activation(
                tmp[:, 0:w], zbufs[zc][:], AF.Relu, bias=shift[:],
                scale=scale[:]
            )
            obuf = fpool.tile([P, 8 * P], BF16, tag="obuf")
            nc.vector.tensor_add(obuf[:, 0:w], tmp[:, 0:w], xtbuf[:, sl])
            nc.sync.dma_start(out_d[:, sl], obuf[:, 0:w])

    nc.compile()
    return nc


# ---------------------------------------------------------------- entry point

def kernel(feature, W, b, gamma, beta, src, dst, _trace=False,
           _skip_collective=False):
    n = feature.shape[0]
    cfg, in_maps = host_prep(feature, W, gamma, beta, src, dst)
    nc = build_program(cfg, skip_collective=_skip_collective)

    def _run():
        # materialize results inside the attempt: device failures surface
        # lazily at array fetch, and must be covered by the retry
        r = run_bass_kernel_spmd(
            nc, in_maps, list(range(cfg["ncores"])), trace=_trace
        )
        outs = [np.asarray(r.results[c]["outt"]) for c in range(cfg["ncores"])]
        return r, outs

    try:
        res, outs = _run()
    except Exception:
        # retries: a previously-wedged device can fail the first attempts
        try:
            res, outs = _run()
        except Exception:
            res, outs = _run()
    full = np.empty((cfg["npad"], D), dtype=np.float32)
    for c in range(cfg["ncores"]):
        full[cfg["rows_of_core"][c]] = outs[c].T.astype(np.float32)
    out = full[:n]
    if _trace:
        return out, res
    return out



# revision 17
# speedup vs baseline: 1.0002x; 1.0002x over previous
"""GCN layer (message passing + linear + BatchNorm + ReLU + residual) on 8 TRN2 cores.

V2 strategy (graph/data parallel, per sharding hint):
  - Nodes (and their incident edges, grouped by dst tile) sharded across 8
    cores; dst tiles DEALT to (slot, core) by descending edge count so all
    cores see near-identical per-slot counts (SPMD shapes are cross-core max).
  - Node table relabeled by a greedy parity-balancing pass (discrepancy-style)
    so even/odd gather lists are balanced per tile; rows stored as bf16 only
    (no hi/lo pair) — full-pipeline rel err ~3e-3 vs the 2e-2 gate.
  - Feature rows gathered via SWDGE dma_gather (256B rows) from a replicated
    DRAM table; calls batched per (group, parity-view) up to the 1024-desc
    SWDGE ring cap (HW-bisected; resizing the ring crashes the device).
  - Per-tile edge lists packed at 16-granularity inside each group region;
    blocks shared by two tiles get one matmul per tile with complementary
    one-hot columns (dstloc = -1 masks foreign slots).
  - Segment-mean fused into the one-hot build: oh = (iota == dstloc) * recip
    via a single DVE tensor_scalar(is_equal, mult) — no separate recip
    multiply and no [128, shard] recip broadcast load.
  - Slots processed in QUADS of <=4 sharing one PSUM bank: one Act h1-copy,
    one 512-wide hi/lo W-matmul pair, one z-copy and one Square per quad
    (amortizes the Act engine's per-op PSUM/SBUF access latency 4x). Quads
    are flat runs over the slot sequence and may span gather groups, so the
    taper tail drains as a single quad.
  - Linear = W @ h1 with hi/lo split W (bf16 pair), h1 = bf16 copy of the
    PSUM aggregate (Act engine; GPSIMD cannot touch PSUM on HW).
  - BN batch stats: Sum(z) accumulated by the Act-engine PSUM->zbuf copy
    (accum_out, zbuf in bf16), Sum(z^2) by an Act Square pass; [128,2]
    AllReduce across cores; b omitted (training-mode BN is invariant to b).
  - Residual features streamed early (overlaps gathers); finalize writes out
    in ~1KB-wide chunks (pairs of quads) so the output DMA pipelines with
    the Act relu + DVE residual-add tail.

  Explored and rejected (cost model + HW experiments, see session notes):
  fp8/int8 message gathers (fp8 fails the 2e-2 gate at full rate and SWDGE
  descriptors have a 256B floor, so sub-256B rows do not reduce modeled DMA
  time; int8 is not a PE matmul dtype), >1024-desc SWDGE calls and ring
  resizing (crash the device), gpsimd ap_gather (0.6 efficiency makes it no
  cheaper than SWDGE desc-gen), indirect_dma_start bulk gathers (HW consumes
  one offset per partition, i.e. 128 rows/call max).
"""

import sys

for _p in ("/opt/trn_rl_repo", "/root/.axon_site/_ro/trn_rl_repo"):
    if _p not in sys.path:
        sys.path.insert(0, _p)

import numpy as np
from contextlib import ExitStack

import concourse.bass as bass
import concourse.bacc as bacc
import concourse.mybir as mybir
import concourse.tile as tile
from concourse.bass_utils import run_bass_kernel_spmd

try:
    from ml_dtypes import bfloat16 as np_bf16
except ImportError:  # jax always ships ml_dtypes
    import jax.numpy as _jnp

    np_bf16 = _jnp.bfloat16

F32 = mybir.dt.float32
BF16 = mybir.dt.bfloat16
I16 = mybir.dt.int16

P = 128          # partitions / tile node count / edge block size
D = 128          # feature dim
NCORES = 8
BN_EPS = 1e-5
GMAX = 1024      # SWDGE ring descriptor cap (HW-bisected: >1024 crashes, and
RING = 16384     # resizing dynamic_dma_scratch_size crashes even small calls)
GTILES = 7       # dst tiles per gather group
TAPER = (4, 2, 2, 1, 1)
MBUFS = 3


def _r16(x):
    return (int(x) + 15) // 16 * 16


# ---------------------------------------------------------------- host prep

def _parity_greedy(n, src, gtile, ntg, npad):
    """Choose a parity bit per node minimizing per-tile even/odd imbalance.

    Returns (par[n] in {0,1}, rank[n] = row within its parity class).
    """
    order = np.argsort(src, kind="stable")
    s_sorted = src[order]
    g_sorted = gtile[order]
    starts = np.searchsorted(s_sorted, np.arange(n + 1))
    outdeg = starts[1:] - starts[:-1]
    nord = np.argsort(-outdeg, kind="stable")

    imb = np.zeros(ntg, dtype=np.int64)
    par = np.zeros(n, dtype=np.int8)
    csize = [0, 0]
    half = npad // 2
    for v in nord:
        lo, hi = starts[v], starts[v + 1]
        if lo == hi:
            p = 0 if csize[0] <= csize[1] else 1
        else:
            tv = g_sorted[lo:hi]
            s = imb[tv].sum()
            if s > 0:
                p = 1
            elif s < 0:
                p = 0
            else:
                p = 0 if csize[0] <= csize[1] else 1
            if csize[p] >= half:          # class full: forced
                p = 1 - p
            np.add.at(imb, tv, 1 - 2 * p)
        par[v] = p
        csize[p] += 1
    rank = np.zeros(n, dtype=np.int64)
    for q in (0, 1):
        idx = np.flatnonzero(par == q)
        rank[idx] = np.arange(idx.shape[0])
    return par, rank


def host_prep(feature, W, gamma, beta, src, dst, ncores=NCORES):
    """Index-only graph preprocessing + data layout. Returns (cfg, in_maps)."""
    n, d = feature.shape
    assert d == D
    shard = -(-n // (ncores * P)) * P          # per-core node count, mult of 128
    npad = shard * ncores
    nt = shard // P                            # dst tiles per core
    ntg = ncores * nt

    src = np.asarray(src, dtype=np.int64)
    dst = np.asarray(dst, dtype=np.int64)

    deg = np.bincount(dst, minlength=npad).astype(np.float64)
    recip = np.where(deg > 0, 1.0 / np.maximum(deg, 1.0), 0.0).astype(np.float32)

    gtile = dst // P                           # global dst tile of each edge
    tile_counts = np.bincount(gtile, minlength=ntg)

    # deal tiles to (slot, core) by descending count: cross-core maxes shrink
    deal = np.argsort(-tile_counts, kind="stable").reshape(nt, ncores)

    # parity balancing + node relabel (table order only; dst untouched)
    par, rank = _parity_greedy(n, src, gtile, ntg, npad)

    # per-edge gather data
    e_par = par[src]                           # which view
    e_idx = rank[src].astype(np.int16)         # row within view
    e_slot = (dst % P).astype(np.float32)      # dst slot within its tile
    e_rec = recip[dst]                         # fold 1/deg into the one-hot

    # edges grouped by global tile
    eorder = np.argsort(gtile, kind="stable")
    tstarts = np.searchsorted(gtile[eorder], np.arange(ntg + 1))

    def tile_edges(g, q):
        ee = eorder[tstarts[g] : tstarts[g + 1]]
        ee = ee[e_par[ee] == q]
        return e_idx[ee], e_slot[ee], e_rec[ee]

    # per (slot, parity) padded length = r16(max over cores)
    cnt = np.zeros((nt, ncores, 2), dtype=np.int64)
    for s in range(nt):
        for c in range(ncores):
            g = deal[s][c]
            for q in (0, 1):
                ee = eorder[tstarts[g] : tstarts[g + 1]]
                cnt[s, c, q] = int((e_par[ee] == q).sum())
    pad_len = np.zeros((nt, 2), dtype=np.int64)
    for s in range(nt):
        for q in (0, 1):
            pad_len[s, q] = _r16(cnt[s, :, q].max())

    # groups of slots; per group: [ev region | od region], regions 128-block
    # aligned at region starts (ev region padded to full blocks). First and
    # last groups are small: compute starts sooner after the first gather,
    # and the serial compute tail after the final gather is short.
    taper = list(TAPER)
    sizes = [2]
    rem = nt - 2 - sum(taper)
    while rem > GTILES:
        sizes.append(GTILES)
        rem -= GTILES
    if rem:
        sizes.append(rem)
    sizes.extend(taper)
    groups = []
    pos0 = 0
    for sz in sizes:
        groups.append(list(range(pos0, pos0 + sz)))
        pos0 += sz
    assert pos0 == nt
    ginfo = []          # per group dict
    tile_prog = [[] for _ in range(nt)]   # per slot: list of (mb, col)
    ncols = 0
    icols = 0
    gbmax = 0
    for gi, g in enumerate(groups):
        # regions rounded to whole 128-blocks: every gathered slot is written
        # (pad indices are 0 -> valid row data; dstloc=-1 masks them), else
        # unwritten SBUF tail bytes (can be NaN) would flow into the matmul
        ev_len = -(-int(sum(pad_len[s, 0] for s in g)) // P) * P
        od_len = -(-int(sum(pad_len[s, 1] for s in g)) // P) * P
        ebg = ev_len // P
        obg = od_len // P
        nblk = ebg + obg
        gbmax = max(gbmax, ebg, obg)

        # per-view gather calls (split at the ring cap)
        gmax_g = GMAX
        def _calls(ln, blk0, ic0):
            out = []
            off = 0
            while ln - off > gmax_g:
                step = (gmax_g // P) * P
                out.append((ic0 + off // 16, step, blk0 + off // P))
                off += step
            if ln > off:
                out.append((ic0 + off // 16, ln - off, blk0 + off // P))
            return out

        ev_calls = _calls(ev_len, 0, icols)
        od_calls = _calls(od_len, 0, icols + ev_len // 16)

        # per-slot region offsets and (view, block, col) program; blocks are
        # view-relative (ev and od live in separate msgs tiles so compute on
        # one view's blocks never waits for the other view's gather)
        col_meta = []   # (slot, q, region_pos0, blk)  for building dl/re later
        for q in (0, 1):
            pos = 0
            for s in g:
                ln = int(pad_len[s, q])
                if ln == 0:
                    continue
                b0 = pos // P
                b1 = -(-(pos + ln) // P)
                for b in range(b0, b1):
                    tile_prog[s].append((q, b, ncols))
                    col_meta.append((s, q, pos, b))
                    ncols += 1
                pos += ln
        ginfo.append(
            {
                "slots": g,
                "ev_len": ev_len,
                "od_len": od_len,
                "ebg": ebg,
                "obg": obg,
                "nblk": nblk,
                "ev_calls": ev_calls,
                "od_calls": od_calls,
                "icol0": icols,
                "col_meta": col_meta,
            }
        )
        icols += ev_len // 16 + od_len // 16

    total_desc = sum(gg["ev_len"] + gg["od_len"] for gg in ginfo)

    # ---- per-core arrays ----
    xpad = np.zeros((npad, D), dtype=np.float32)
    xpad[:n] = np.asarray(feature, dtype=np.float32)
    fbf = xpad[:n].astype(np_bf16)
    xsbf = np.zeros((npad // 2, 2, D), dtype=np_bf16)
    for q in (0, 1):
        idxq = np.flatnonzero(par == q)
        xsbf[rank[idxq], q, :] = fbf[idxq]

    wt = np.ascontiguousarray(np.asarray(W, dtype=np.float32).T)
    wthi = wt.astype(np_bf16)
    wtlo = (wt - wthi.astype(np.float32)).astype(np_bf16)
    wtcat = np.concatenate([wthi, wtlo], axis=1)             # [128, 256] bf16

    iotab = np.ascontiguousarray(
        np.broadcast_to(np.arange(P, dtype=np.float32), (P, P)).astype(np_bf16)
    )
    gb = np.stack(
        [np.asarray(gamma, np.float32), np.asarray(beta, np.float32)], axis=1
    )

    col_desc = []  # (gi, s, q, pos0, b) in sequential col-id order
    for gi, gg in enumerate(ginfo):
        for s, q, pos0, b in gg["col_meta"]:
            col_desc.append((gi, s, q, pos0, b))
    assert len(col_desc) == ncols

    in_maps = [None] * ncores
    rows_of_core = []
    for c in range(ncores):
        rows = np.concatenate(
            [np.arange(deal[s][c] * P, deal[s][c] * P + P) for s in range(nt)]
        )
        rows_of_core.append(rows)
        idx_cols = np.zeros((16, icols), dtype=np.int16)
        dl = np.full((P, ncols), -1.0, dtype=np.float32)
        re = np.zeros((P, ncols), dtype=np.float32)

        for gg in ginfo:
            for q in (0, 1):
                ic = gg["icol0"] + (gg["ev_len"] // 16 if q == 1 else 0)
                for s in gg["slots"]:
                    ln = int(pad_len[s, q])
                    if ln == 0:
                        continue
                    ii, _, _ = tile_edges(deal[s][c], q)
                    vals = np.zeros(ln, dtype=np.int16)
                    vals[: ii.shape[0]] = ii
                    idx_cols[:, ic : ic + ln // 16] = vals.reshape(ln // 16, 16).T
                    ic += ln // 16

        # per-column dstloc/recip: column (s, q, pos0, b): block b covers
        # region positions [b*P, (b+1)*P); the slot's edges occupy
        # [pos0, pos0+cnt(c,s,q)) with dl = slot, re = recip
        for colid, (gi, s, q, pos0, b) in enumerate(col_desc):
            _, ss, rr = tile_edges(deal[s][c], q)
            m = ss.shape[0]
            lo = b * P
            dlcol = np.full(P, -1.0, dtype=np.float32)
            recol = np.zeros(P, dtype=np.float32)
            # positions of this slot's real edges inside this block
            p0 = max(pos0, lo)
            p1 = min(pos0 + m, lo + P)
            if p1 > p0:
                dlcol[p0 - lo : p1 - lo] = ss[p0 - pos0 : p1 - pos0]
                recol[p0 - lo : p1 - lo] = rr[p0 - pos0 : p1 - pos0]
            dl[:, colid] = dlcol
            re[:, colid] = recol

        xt = np.ascontiguousarray(xpad[rows].T.astype(np_bf16))  # [128, shard] bf16

        in_maps[c] = {
            "xsbf": np.ascontiguousarray(xsbf),
            "xt": xt,
            "idx": np.ascontiguousarray(np.tile(idx_cols, (8, 1))),
            "dl": np.ascontiguousarray(dl),
            "re": np.ascontiguousarray(re),
            "wt": np.ascontiguousarray(wtcat),
            "iotab": iotab,
            "gb": np.ascontiguousarray(gb),
        }

    cfg = {
        "n": n,
        "npad": npad,
        "shard": shard,
        "nt": nt,
        "ncores": ncores,
        "icols": icols,
        "ncols": ncols,
        "gbmax": gbmax,
        "ginfo": ginfo,
        "tile_prog": tile_prog,
        "total_desc": total_desc,
        "rows_of_core": rows_of_core,
    }
    return cfg, in_maps


# ---------------------------------------------------------------- device program

def build_program(cfg, skip_collective=False):
    ncores = cfg["ncores"]
    shard, nt = cfg["shard"], cfg["nt"]
    npad = cfg["npad"]
    icols, ncols = cfg["icols"], cfg["ncols"]
    gbmax = cfg["gbmax"]
    ginfo = cfg["ginfo"]
    tile_prog = cfg["tile_prog"]
    inv_n = 1.0 / cfg["n"]

    nc = bacc.Bacc("TRN2", target_bir_lowering=False, debug=False,
                   num_devices=ncores, dynamic_dma_scratch_size=RING)

    xsbf = nc.declare_dram_parameter("xsbf", [npad // 2, 2, D], BF16, False)
    xt_d = nc.declare_dram_parameter("xt", [P, shard], BF16, False)
    idx_d = nc.declare_dram_parameter("idx", [P, icols], I16, False)
    dl_d = nc.declare_dram_parameter("dl", [P, ncols], F32, False)
    re_d = nc.declare_dram_parameter("re", [P, ncols], F32, False)
    wt_d = nc.declare_dram_parameter("wt", [P, 2 * D], BF16, False)
    io_d = nc.declare_dram_parameter("iotab", [P, P], BF16, False)
    gb_d = nc.declare_dram_parameter("gb", [P, 2], F32, False)
    out_d = nc.declare_dram_parameter("outt", [P, shard], BF16, True)

    AL = mybir.AluOpType
    AF = mybir.ActivationFunctionType

    with ExitStack() as ctx:
        tc = ctx.enter_context(tile.TileContext(nc))
        const = ctx.enter_context(tc.tile_pool(name="const", bufs=1))
        mpool = ctx.enter_context(tc.tile_pool(name="msgs", bufs=MBUFS))
        ohpool = ctx.enter_context(tc.tile_pool(name="oh", bufs=20))
        hpool = ctx.enter_context(tc.tile_pool(name="h1", bufs=3))
        scpool = ctx.enter_context(tc.tile_pool(name="scratch", bufs=2))
        fpool = ctx.enter_context(tc.tile_pool(name="fin", bufs=4))
        stat = ctx.enter_context(tc.tile_pool(name="stat", bufs=1))
        apool = ctx.enter_context(tc.tile_pool(name="aggps", bufs=2, space="PSUM"))
        zpool = ctx.enter_context(tc.tile_pool(name="zps", bufs=2, space="PSUM"))
        dram = ctx.enter_context(tc.tile_pool(name="dram", bufs=2, space="DRAM"))

        ic_split = ginfo[0]["icol0"] + (ginfo[0]["ev_len"] + ginfo[0]["od_len"]) // 16
        idx_s0 = const.tile([P, ic_split], I16)
        idx_s1 = const.tile([P, icols - ic_split], I16)
        dl_s = const.tile([P, ncols], F32)
        re_s = const.tile([P, ncols], F32)
        wt_s = const.tile([P, 2 * D], BF16)
        io_s = const.tile([P, P], BF16)
        gb_s = const.tile([P, 2], F32)
        # slots processed in QUADS of <=4 (one PSUM bank): one Act h1-copy,
        # one wide z-matmul pair, one z-copy and one Square per quad instead
        # of per slot -- the Act engine's per-op SBUF/PSUM access latency is
        # amortized 4x. Quads are flat runs over the slot sequence and MAY
        # span gather groups (dep tracking orders them after the gathers they
        # read; the taper tail then drains as one quad, not 3-4 tiny ones).
        quads = []                     # [slots]
        i_ = 0
        while i_ < nt:
            take = min(4, nt - i_)
            if nt - i_ == 5:
                take = 3               # split 5 as 3+2, not 4+1
            quads.append(list(range(i_, i_ + take)))
            i_ += take
        nquads = len(quads)
        group_of_slot = {}
        for gi_, gg_ in enumerate(ginfo):
            for s_ in gg_["slots"]:
                group_of_slot[s_] = gi_
        # a quad is computed once the group of its LAST slot has gathered
        quads_done_in_group = {gi_: [] for gi_ in range(len(ginfo))}
        for qi_, qs_ in enumerate(quads):
            quads_done_in_group[group_of_slot[qs_[-1]]].append(qi_)
        # finalize chunks = pairs of quads (~8 tiles / 1 KB-wide Act ops);
        # per-chunk zbuf tiles keep the finalize pipeline from serializing
        # against one big buffer via tile-granular deps
        zchunks = [list(range(k, min(k + 2, nquads))) for k in range(0, nquads, 2)]
        zc_of_quad = {}
        for zc_, ql_ in enumerate(zchunks):
            off_ = 0
            for qi_ in ql_:
                zc_of_quad[qi_] = (zc_, off_)
                off_ += len(quads[qi_]) * P
        zwidth = [sum(len(quads[qi_]) for qi_ in ql_) * P for ql_ in zchunks]
        zbufs = [const.tile([P, zwidth[zc_]], BF16, name=f"zbuf{zc_}")
                 for zc_ in range(len(zchunks))]
        xtbuf = const.tile([P, shard], BF16)
        sacc = const.tile([P, nquads], F32)
        qacc = const.tile([P, nquads], F32)

        # first group's idx columns first (its gathers start ~1.5us earlier),
        # then the rest; xt deferred to the compute tail after the last gather
        # so it never delays gather descriptors
        nc.sync.dma_start(idx_s0[:], idx_d[:, 0:ic_split])
        nc.sync.dma_start(idx_s1[:], idx_d[:, ic_split:icols])
        nc.sync.dma_start(io_s[:], io_d[:])
        nc.sync.dma_start(dl_s[:], dl_d[:])
        nc.sync.dma_start(re_s[:], re_d[:])
        nc.sync.dma_start(wt_s[:], wt_d[:])
        nc.sync.dma_start(gb_s[:], gb_d[:])

        # dummy Sqrt: selects the act-function set covering Copy, Square,
        # Relu AND Sqrt ("sqrt_and_others"), so no mid-stream table reload
        # stalls the Act engine before the BN scalar chain
        epsb = stat.tile([P, 1], F32)
        nc.vector.memset(epsb[:], float(BN_EPS))
        warm = stat.tile([P, 1], F32)
        nc.scalar.activation(warm[:], epsb[:], AF.Sqrt)

        ev_view = xsbf[:, 0, :]   # [npad//2, D] stride 2*D
        od_view = xsbf[:, 1, :]

        ngroups = len(ginfo)
        group_msgs = {}
        for gi, gg in enumerate(ginfo):
            msgse = mpool.tile([P, gbmax, D], BF16, tag="msgse")
            msgso = mpool.tile([P, gbmax, D], BF16, tag="msgso")
            group_msgs[gi] = (msgse, msgso)
            for view, mt, calls in ((ev_view, msgse, gg["ev_calls"]),
                                    (od_view, msgso, gg["od_calls"])):
                for ic0, ln, blk0 in calls:
                    if ic0 < ic_split:
                        idx_ap = idx_s0[:, ic0 : ic0 + ln // 16]
                    else:
                        idx_ap = idx_s1[:, ic0 - ic_split : ic0 - ic_split + ln // 16]
                    nc.gpsimd.dma_gather(
                        mt[:, blk0 : blk0 + (-(-ln // P)), :],
                        view,
                        idx_ap,
                        ln,
                        ln,
                        elem_size=D,
                        elem_step=2 * D,
                    )

            for qi in quads_done_in_group[gi]:
                qs = quads[qi]
                w = len(qs) * P
                aggq = apool.tile([P, 4 * P], F32, tag="agg")
                for j, s in enumerate(qs):
                    prog = tile_prog[s]
                    msgs_q = group_msgs[group_of_slot[s]]
                    nk = len(prog)
                    for k, (q, mb, col) in enumerate(prog):
                        oh = ohpool.tile([P, P], BF16, tag="oh")
                        nc.vector.tensor_scalar(
                            oh[:], io_s[:],
                            dl_s[:, col : col + 1], re_s[:, col : col + 1],
                            AL.is_equal, AL.mult,
                        )
                        nc.tensor.matmul(
                            aggq[:, j * P : (j + 1) * P],
                            msgs_q[q][:, mb, :], oh[:],
                            start=(k == 0), stop=(k == nk - 1),
                        )

                h1q = hpool.tile([P, 4 * P], BF16, tag="h1")
                nc.scalar.activation(h1q[:, 0:w], aggq[:, 0:w], AF.Copy)

                zq = zpool.tile([P, 4 * P], F32, tag="zp")
                nc.tensor.matmul(zq[:, 0:w], wt_s[:, 0:D], h1q[:, 0:w],
                                 start=True, stop=False)
                nc.tensor.matmul(zq[:, 0:w], wt_s[:, D : 2 * D], h1q[:, 0:w],
                                 start=False, stop=True)

                # PSUM -> zbuf copy with Sum(z) accumulation; Square pass for
                # Sum(z^2). Both on the Act engine (PE/DVE stay free).
                zc, zoff = zc_of_quad[qi]
                nc.scalar.activation(
                    zbufs[zc][:, zoff : zoff + w], zq[:, 0:w],
                    AF.Copy, accum_out=sacc[:, qi : qi + 1],
                )
                sq = scpool.tile([P, 4 * P], BF16, tag="sq")
                nc.scalar.activation(
                    sq[:, 0:w], zq[:, 0:w], AF.Square,
                    accum_out=qacc[:, qi : qi + 1]
                )

        # residual stream issued on the Pool SWDGE queue: its descriptor gen
        # queues behind the final gather's, so the transfer lands in the
        # post-gather tail instead of ahead of the last gathers (the idle SP
        # engine would otherwise request the DMA unit early and push the
        # gather stream's end out by the full xt transfer time)
        nc.gpsimd.dma_start(xtbuf[:], xt_d[:])

        # ---- BatchNorm statistics (global over all cores) ----
        ssum = stat.tile([P, 2], F32)
        nc.vector.tensor_reduce(
            ssum[:, 0:1], sacc[:], axis=mybir.AxisListType.X, op=AL.add
        )
        nc.vector.tensor_reduce(
            ssum[:, 1:2], qacc[:], axis=mybir.AxisListType.X, op=AL.add
        )

        tot = stat.tile([P, 2], F32)
        if skip_collective:
            tot = ssum
        else:
            cin = dram.tile([P, 2], F32)
            cout = dram.tile([P, 2], F32)
            nc.gpsimd.dma_start(cin[:], ssum[:])
            nc.gpsimd.collective_compute(
                "AllReduce",
                AL.add,
                replica_groups=[list(range(ncores))],
                ins=[cin.opt()],
                outs=[cout.opt()],
            )
            nc.gpsimd.dma_start(tot[:], cout[:])

        # scale = gamma / sqrt(var+eps); shift = beta - mu*scale
        m2 = stat.tile([P, 2], F32)
        nc.vector.tensor_scalar(m2[:], tot[:], inv_n, None, AL.mult)
        mu = m2[:, 0:1]
        var = stat.tile([P, 1], F32)
        nc.vector.tensor_mul(var[:], m2[:, 0:1], m2[:, 0:1])
        nc.vector.tensor_sub(var[:], m2[:, 1:2], var[:])
        sd = stat.tile([P, 1], F32)
        nc.scalar.activation(sd[:], var[:], AF.Sqrt, bias=epsb[:])
        inv = stat.tile([P, 1], F32)
        nc.vector.reciprocal(inv[:], sd[:])
        scale = stat.tile([P, 1], F32)
        nc.vector.tensor_mul(scale[:], gb_s[:, 0:1], inv[:])
        shift = stat.tile([P, 1], F32)
        nc.vector.tensor_mul(shift[:], m2[:, 0:1], scale[:])
        nc.vector.tensor_sub(shift[:], gb_s[:, 1:2], shift[:])

        # ---- finalize: out = x + relu(z*scale + shift); xt was preloaded,
        # output written per chunk so the store DMA overlaps the compute ----
        # the first two chunks run on a pure-DVE path (relu(a)+x computed as
        # max(a+x, x); bf16 tensor_scalar hits the 4x DVE mode) while the Act
        # engine streams the remaining relu chunks in parallel -- the two
        # finalize streams overlap instead of serializing on Act
        for zc, ql in enumerate(zchunks):
            w = zwidth[zc]
            c0 = quads[ql[0]][0] * P
            sl = slice(c0, c0 + w)
            if zc < 2:
                t1 = fpool.tile([P, 8 * P], BF16, tag="fin")
                nc.vector.tensor_scalar(
                    t1[:, 0:w], zbufs[zc][:], scale[:], shift[:],
                    AL.mult, AL.add,
                )
                t2 = fpool.tile([P, 8 * P], BF16, tag="fin2")
                nc.vector.tensor_add(t2[:, 0:w], t1[:, 0:w], xtbuf[:, sl])
                obuf = fpool.tile([P, 8 * P], BF16, tag="obuf")
                nc.vector.tensor_tensor(
                    out=obuf[:, 0:w], in0=t2[:, 0:w], in1=xtbuf[:, sl],
                    op=AL.max,
                )
            else:
                tmp = fpool.tile([P, 8 * P], BF16, tag="fin")
                nc.scalar.activation(
                    tmp[:, 0:w], zbufs[zc][:], AF.Relu, bias=shift[:],
                    scale=scale[:]
                )
                obuf = fpool.tile([P, 8 * P], BF16, tag="obuf")
                nc.vector.tensor_add(obuf[:, 0:w], tmp[:, 0:w], xtbuf[:, sl])
            nc.sync.dma_start(out_d[:, sl], obuf[:, 0:w])

    nc.compile()
    return nc


# ---------------------------------------------------------------- entry point

def kernel(feature, W, b, gamma, beta, src, dst, _trace=False,
           _skip_collective=False):
    n = feature.shape[0]
    cfg, in_maps = host_prep(feature, W, gamma, beta, src, dst)
    nc = build_program(cfg, skip_collective=_skip_collective)

    def _run():
        # materialize results inside the attempt: device failures surface
        # lazily at array fetch, and must be covered by the retry
        r = run_bass_kernel_spmd(
            nc, in_maps, list(range(cfg["ncores"])), trace=_trace
        )
        outs = [np.asarray(r.results[c]["outt"]) for c in range(cfg["ncores"])]
        return r, outs

    try:
        res, outs = _run()
    except Exception:
        # retries: a previously-wedged device can fail the first attempts
        try:
            res, outs = _run()
        except Exception:
            res, outs = _run()
    full = np.empty((cfg["npad"], D), dtype=np.float32)
    for c in range(cfg["ncores"]):
        full[cfg["rows_of_core"][c]] = outs[c].T.astype(np.float32)
    out = full[:n]
    if _trace:
        return out, res
    return out



# revision 19
# speedup vs baseline: 1.0042x; 1.0040x over previous
"""GCN layer (message passing + linear + BatchNorm + ReLU + residual) on 8 TRN2 cores.

V2 strategy (graph/data parallel, per sharding hint):
  - Nodes (and their incident edges, grouped by dst tile) sharded across 8
    cores; dst tiles DEALT to (slot, core) by descending edge count so all
    cores see near-identical per-slot counts (SPMD shapes are cross-core max).
  - Node table relabeled by a greedy parity-balancing pass (discrepancy-style)
    so even/odd gather lists are balanced per tile; rows stored as bf16 only
    (no hi/lo pair) — full-pipeline rel err ~3e-3 vs the 2e-2 gate.
  - Feature rows gathered via SWDGE dma_gather (256B rows) from a replicated
    DRAM table; calls batched per (group, parity-view) up to the 1024-desc
    SWDGE ring cap (HW-bisected; resizing the ring crashes the device).
  - Per-tile edge lists packed at 16-granularity inside each group region;
    blocks shared by two tiles get one matmul per tile with complementary
    one-hot columns (dstloc = -1 masks foreign slots).
  - Segment-mean fused into the one-hot build: oh = (iota == dstloc) * recip
    via a single DVE tensor_scalar(is_equal, mult) — no separate recip
    multiply and no [128, shard] recip broadcast load.
  - Slots processed in QUADS of <=4 sharing one PSUM bank: one Act h1-copy,
    one 512-wide hi/lo W-matmul pair, one z-copy and one Square per quad
    (amortizes the Act engine's per-op PSUM/SBUF access latency 4x). Quads
    are flat runs over the slot sequence and may span gather groups, so the
    taper tail drains as a single quad.
  - Linear = W @ h1 with hi/lo split W (bf16 pair), h1 = bf16 copy of the
    PSUM aggregate (Act engine; GPSIMD cannot touch PSUM on HW).
  - BN batch stats: Sum(z) accumulated by the Act-engine PSUM->zbuf copy
    (accum_out, zbuf in bf16), Sum(z^2) by an Act Square pass; [128,2]
    AllReduce across cores; b omitted (training-mode BN is invariant to b).
  - Residual features streamed early (overlaps gathers); finalize writes out
    in ~1KB-wide chunks (pairs of quads). The first two chunks take a pure
    DVE path (relu(a)+x as max(a+x, x), 4x-mode tensor_scalar) while the Act
    engine streams the remaining relu chunks in parallel, so the two
    finalize streams overlap instead of serializing on Act.

  Explored and rejected (cost model + HW experiments, see session notes):
  fp8/int8 message gathers (fp8 fails the 2e-2 gate at full rate and SWDGE
  descriptors have a 256B floor, so sub-256B rows do not reduce modeled DMA
  time; int8 is not a PE matmul dtype), >1024-desc SWDGE calls and ring
  resizing (crash the device), gpsimd ap_gather (0.6 efficiency makes it no
  cheaper than SWDGE desc-gen), indirect_dma_start bulk gathers (HW consumes
  one offset per partition, i.e. 128 rows/call max).
"""

import sys

for _p in ("/opt/trn_rl_repo", "/root/.axon_site/_ro/trn_rl_repo"):
    if _p not in sys.path:
        sys.path.insert(0, _p)

import numpy as np
from contextlib import ExitStack

import concourse.bass as bass
import concourse.bacc as bacc
import concourse.mybir as mybir
import concourse.tile as tile
from concourse.bass_utils import run_bass_kernel_spmd

try:
    from ml_dtypes import bfloat16 as np_bf16
except ImportError:  # jax always ships ml_dtypes
    import jax.numpy as _jnp

    np_bf16 = _jnp.bfloat16

F32 = mybir.dt.float32
BF16 = mybir.dt.bfloat16
I16 = mybir.dt.int16

P = 128          # partitions / tile node count / edge block size
D = 128          # feature dim
NCORES = 8
BN_EPS = 1e-5
GMAX = 1024      # SWDGE ring descriptor cap (HW-bisected: >1024 crashes, and
RING = 16384     # resizing dynamic_dma_scratch_size crashes even small calls)
GTILES = 7       # dst tiles per gather group
TAPER = (4, 2, 2, 1, 1)
MBUFS = 3


def _r16(x):
    return (int(x) + 15) // 16 * 16


# ---------------------------------------------------------------- host prep

def _parity_greedy(n, src, gtile, ntg, npad):
    """Choose a parity bit per node minimizing per-tile even/odd imbalance.

    Returns (par[n] in {0,1}, rank[n] = row within its parity class).
    """
    order = np.argsort(src, kind="stable")
    s_sorted = src[order]
    g_sorted = gtile[order]
    starts = np.searchsorted(s_sorted, np.arange(n + 1))
    outdeg = starts[1:] - starts[:-1]
    nord = np.argsort(-outdeg, kind="stable")

    imb = np.zeros(ntg, dtype=np.int64)
    par = np.zeros(n, dtype=np.int8)
    csize = [0, 0]
    half = npad // 2
    for v in nord:
        lo, hi = starts[v], starts[v + 1]
        if lo == hi:
            p = 0 if csize[0] <= csize[1] else 1
        else:
            tv = g_sorted[lo:hi]
            s = imb[tv].sum()
            if s > 0:
                p = 1
            elif s < 0:
                p = 0
            else:
                p = 0 if csize[0] <= csize[1] else 1
            if csize[p] >= half:          # class full: forced
                p = 1 - p
            np.add.at(imb, tv, 1 - 2 * p)
        par[v] = p
        csize[p] += 1
    rank = np.zeros(n, dtype=np.int64)
    for q in (0, 1):
        idx = np.flatnonzero(par == q)
        rank[idx] = np.arange(idx.shape[0])
    return par, rank


def host_prep(feature, W, gamma, beta, src, dst, ncores=NCORES):
    """Index-only graph preprocessing + data layout. Returns (cfg, in_maps)."""
    n, d = feature.shape
    assert d == D
    shard = -(-n // (ncores * P)) * P          # per-core node count, mult of 128
    npad = shard * ncores
    nt = shard // P                            # dst tiles per core
    ntg = ncores * nt

    src = np.asarray(src, dtype=np.int64)
    dst = np.asarray(dst, dtype=np.int64)

    deg = np.bincount(dst, minlength=npad).astype(np.float64)
    recip = np.where(deg > 0, 1.0 / np.maximum(deg, 1.0), 0.0).astype(np.float32)

    gtile = dst // P                           # global dst tile of each edge
    tile_counts = np.bincount(gtile, minlength=ntg)

    # deal tiles to (slot, core) by descending count: cross-core maxes shrink
    deal = np.argsort(-tile_counts, kind="stable").reshape(nt, ncores)

    # parity balancing + node relabel (table order only; dst untouched)
    par, rank = _parity_greedy(n, src, gtile, ntg, npad)

    # per-edge gather data
    e_par = par[src]                           # which view
    e_idx = rank[src].astype(np.int16)         # row within view
    e_slot = (dst % P).astype(np.float32)      # dst slot within its tile
    e_rec = recip[dst]                         # fold 1/deg into the one-hot

    # edges grouped by global tile
    eorder = np.argsort(gtile, kind="stable")
    tstarts = np.searchsorted(gtile[eorder], np.arange(ntg + 1))

    def tile_edges(g, q):
        ee = eorder[tstarts[g] : tstarts[g + 1]]
        ee = ee[e_par[ee] == q]
        return e_idx[ee], e_slot[ee], e_rec[ee]

    # per (slot, parity) padded length = r16(max over cores)
    cnt = np.zeros((nt, ncores, 2), dtype=np.int64)
    for s in range(nt):
        for c in range(ncores):
            g = deal[s][c]
            for q in (0, 1):
                ee = eorder[tstarts[g] : tstarts[g + 1]]
                cnt[s, c, q] = int((e_par[ee] == q).sum())
    pad_len = np.zeros((nt, 2), dtype=np.int64)
    for s in range(nt):
        for q in (0, 1):
            pad_len[s, q] = _r16(cnt[s, :, q].max())

    # groups of slots; per group: [ev region | od region], regions 128-block
    # aligned at region starts (ev region padded to full blocks). First and
    # last groups are small: compute starts sooner after the first gather,
    # and the serial compute tail after the final gather is short.
    taper = list(TAPER)
    sizes = [2]
    rem = nt - 2 - sum(taper)
    while rem > GTILES:
        sizes.append(GTILES)
        rem -= GTILES
    if rem:
        sizes.append(rem)
    sizes.extend(taper)
    groups = []
    pos0 = 0
    for sz in sizes:
        groups.append(list(range(pos0, pos0 + sz)))
        pos0 += sz
    assert pos0 == nt
    ginfo = []          # per group dict
    tile_prog = [[] for _ in range(nt)]   # per slot: list of (mb, col)
    ncols = 0
    icols = 0
    gbmax = 0
    for gi, g in enumerate(groups):
        # regions rounded to whole 128-blocks: every gathered slot is written
        # (pad indices are 0 -> valid row data; dstloc=-1 masks them), else
        # unwritten SBUF tail bytes (can be NaN) would flow into the matmul
        ev_len = -(-int(sum(pad_len[s, 0] for s in g)) // P) * P
        od_len = -(-int(sum(pad_len[s, 1] for s in g)) // P) * P
        ebg = ev_len // P
        obg = od_len // P
        nblk = ebg + obg
        gbmax = max(gbmax, ebg, obg)

        # per-view gather calls (split at the ring cap)
        gmax_g = GMAX
        def _calls(ln, blk0, ic0):
            out = []
            off = 0
            while ln - off > gmax_g:
                step = (gmax_g // P) * P
                out.append((ic0 + off // 16, step, blk0 + off // P))
                off += step
            if ln > off:
                out.append((ic0 + off // 16, ln - off, blk0 + off // P))
            return out

        ev_calls = _calls(ev_len, 0, icols)
        od_calls = _calls(od_len, 0, icols + ev_len // 16)

        # per-slot region offsets and (view, block, col) program; blocks are
        # view-relative (ev and od live in separate msgs tiles so compute on
        # one view's blocks never waits for the other view's gather)
        col_meta = []   # (slot, q, region_pos0, blk)  for building dl/re later
        for q in (0, 1):
            pos = 0
            for s in g:
                ln = int(pad_len[s, q])
                if ln == 0:
                    continue
                b0 = pos // P
                b1 = -(-(pos + ln) // P)
                for b in range(b0, b1):
                    tile_prog[s].append((q, b, ncols))
                    col_meta.append((s, q, pos, b))
                    ncols += 1
                pos += ln
        ginfo.append(
            {
                "slots": g,
                "ev_len": ev_len,
                "od_len": od_len,
                "ebg": ebg,
                "obg": obg,
                "nblk": nblk,
                "ev_calls": ev_calls,
                "od_calls": od_calls,
                "icol0": icols,
                "col_meta": col_meta,
            }
        )
        icols += ev_len // 16 + od_len // 16

    total_desc = sum(gg["ev_len"] + gg["od_len"] for gg in ginfo)

    # ---- per-core arrays ----
    xpad = np.zeros((npad, D), dtype=np.float32)
    xpad[:n] = np.asarray(feature, dtype=np.float32)
    fbf = xpad[:n].astype(np_bf16)
    xsbf = np.zeros((npad // 2, 2, D), dtype=np_bf16)
    for q in (0, 1):
        idxq = np.flatnonzero(par == q)
        xsbf[rank[idxq], q, :] = fbf[idxq]

    wt = np.ascontiguousarray(np.asarray(W, dtype=np.float32).T)
    wthi = wt.astype(np_bf16)
    wtlo = (wt - wthi.astype(np.float32)).astype(np_bf16)
    wtcat = np.concatenate([wthi, wtlo], axis=1)             # [128, 256] bf16

    iotab = np.ascontiguousarray(
        np.broadcast_to(np.arange(P, dtype=np.float32), (P, P)).astype(np_bf16)
    )
    gb = np.stack(
        [np.asarray(gamma, np.float32), np.asarray(beta, np.float32)], axis=1
    )

    col_desc = []  # (gi, s, q, pos0, b) in sequential col-id order
    for gi, gg in enumerate(ginfo):
        for s, q, pos0, b in gg["col_meta"]:
            col_desc.append((gi, s, q, pos0, b))
    assert len(col_desc) == ncols

    in_maps = [None] * ncores
    rows_of_core = []
    for c in range(ncores):
        rows = np.concatenate(
            [np.arange(deal[s][c] * P, deal[s][c] * P + P) for s in range(nt)]
        )
        rows_of_core.append(rows)
        idx_cols = np.zeros((16, icols), dtype=np.int16)
        dl = np.full((P, ncols), -1.0, dtype=np.float32)
        re = np.zeros((P, ncols), dtype=np.float32)

        for gg in ginfo:
            for q in (0, 1):
                ic = gg["icol0"] + (gg["ev_len"] // 16 if q == 1 else 0)
                for s in gg["slots"]:
                    ln = int(pad_len[s, q])
                    if ln == 0:
                        continue
                    ii, _, _ = tile_edges(deal[s][c], q)
                    vals = np.zeros(ln, dtype=np.int16)
                    vals[: ii.shape[0]] = ii
                    idx_cols[:, ic : ic + ln // 16] = vals.reshape(ln // 16, 16).T
                    ic += ln // 16

        # per-column dstloc/recip: column (s, q, pos0, b): block b covers
        # region positions [b*P, (b+1)*P); the slot's edges occupy
        # [pos0, pos0+cnt(c,s,q)) with dl = slot, re = recip
        for colid, (gi, s, q, pos0, b) in enumerate(col_desc):
            _, ss, rr = tile_edges(deal[s][c], q)
            m = ss.shape[0]
            lo = b * P
            dlcol = np.full(P, -1.0, dtype=np.float32)
            recol = np.zeros(P, dtype=np.float32)
            # positions of this slot's real edges inside this block
            p0 = max(pos0, lo)
            p1 = min(pos0 + m, lo + P)
            if p1 > p0:
                dlcol[p0 - lo : p1 - lo] = ss[p0 - pos0 : p1 - pos0]
                recol[p0 - lo : p1 - lo] = rr[p0 - pos0 : p1 - pos0]
            dl[:, colid] = dlcol
            re[:, colid] = recol

        xt = np.ascontiguousarray(xpad[rows].T.astype(np_bf16))  # [128, shard] bf16

        in_maps[c] = {
            "xsbf": np.ascontiguousarray(xsbf),
            "xt": xt,
            "idx": np.ascontiguousarray(np.tile(idx_cols, (8, 1))),
            "dl": np.ascontiguousarray(dl),
            "re": np.ascontiguousarray(re),
            "wt": np.ascontiguousarray(wtcat),
            "iotab": iotab,
            "gb": np.ascontiguousarray(gb),
        }

    cfg = {
        "n": n,
        "npad": npad,
        "shard": shard,
        "nt": nt,
        "ncores": ncores,
        "icols": icols,
        "ncols": ncols,
        "gbmax": gbmax,
        "ginfo": ginfo,
        "tile_prog": tile_prog,
        "total_desc": total_desc,
        "rows_of_core": rows_of_core,
    }
    return cfg, in_maps


# ---------------------------------------------------------------- device program

def build_program(cfg, skip_collective=False):
    ncores = cfg["ncores"]
    shard, nt = cfg["shard"], cfg["nt"]
    npad = cfg["npad"]
    icols, ncols = cfg["icols"], cfg["ncols"]
    gbmax = cfg["gbmax"]
    ginfo = cfg["ginfo"]
    tile_prog = cfg["tile_prog"]
    inv_n = 1.0 / cfg["n"]

    nc = bacc.Bacc("TRN2", target_bir_lowering=False, debug=False,
                   num_devices=ncores, dynamic_dma_scratch_size=RING)

    xsbf = nc.declare_dram_parameter("xsbf", [npad // 2, 2, D], BF16, False)
    xt_d = nc.declare_dram_parameter("xt", [P, shard], BF16, False)
    idx_d = nc.declare_dram_parameter("idx", [P, icols], I16, False)
    dl_d = nc.declare_dram_parameter("dl", [P, ncols], F32, False)
    re_d = nc.declare_dram_parameter("re", [P, ncols], F32, False)
    wt_d = nc.declare_dram_parameter("wt", [P, 2 * D], BF16, False)
    io_d = nc.declare_dram_parameter("iotab", [P, P], BF16, False)
    gb_d = nc.declare_dram_parameter("gb", [P, 2], F32, False)
    out_d = nc.declare_dram_parameter("outt", [P, shard], BF16, True)

    AL = mybir.AluOpType
    AF = mybir.ActivationFunctionType

    with ExitStack() as ctx:
        tc = ctx.enter_context(tile.TileContext(nc))
        const = ctx.enter_context(tc.tile_pool(name="const", bufs=1))
        mpool = ctx.enter_context(tc.tile_pool(name="msgs", bufs=MBUFS))
        ohpool = ctx.enter_context(tc.tile_pool(name="oh", bufs=20))
        hpool = ctx.enter_context(tc.tile_pool(name="h1", bufs=3))
        scpool = ctx.enter_context(tc.tile_pool(name="scratch", bufs=2))
        fpool = ctx.enter_context(tc.tile_pool(name="fin", bufs=4))
        stat = ctx.enter_context(tc.tile_pool(name="stat", bufs=1))
        apool = ctx.enter_context(tc.tile_pool(name="aggps", bufs=2, space="PSUM"))
        zpool = ctx.enter_context(tc.tile_pool(name="zps", bufs=2, space="PSUM"))
        dram = ctx.enter_context(tc.tile_pool(name="dram", bufs=2, space="DRAM"))

        ic_split = ginfo[0]["icol0"] + (ginfo[0]["ev_len"] + ginfo[0]["od_len"]) // 16
        idx_s0 = const.tile([P, ic_split], I16)
        idx_s1 = const.tile([P, icols - ic_split], I16)
        dl_s = const.tile([P, ncols], F32)
        re_s = const.tile([P, ncols], F32)
        wt_s = const.tile([P, 2 * D], BF16)
        io_s = const.tile([P, P], BF16)
        gb_s = const.tile([P, 2], F32)
        # slots processed in QUADS of <=4 (one PSUM bank): one Act h1-copy,
        # one wide z-matmul pair, one z-copy and one Square per quad instead
        # of per slot -- the Act engine's per-op SBUF/PSUM access latency is
        # amortized 4x. Quads are flat runs over the slot sequence and MAY
        # span gather groups (dep tracking orders them after the gathers they
        # read; the taper tail then drains as one quad, not 3-4 tiny ones).
        quads = []                     # [slots]
        i_ = 0
        while i_ < nt:
            take = min(4, nt - i_)
            if nt - i_ == 5:
                take = 3               # split 5 as 3+2, not 4+1
            quads.append(list(range(i_, i_ + take)))
            i_ += take
        nquads = len(quads)
        group_of_slot = {}
        for gi_, gg_ in enumerate(ginfo):
            for s_ in gg_["slots"]:
                group_of_slot[s_] = gi_
        # a quad is computed once the group of its LAST slot has gathered
        quads_done_in_group = {gi_: [] for gi_ in range(len(ginfo))}
        for qi_, qs_ in enumerate(quads):
            quads_done_in_group[group_of_slot[qs_[-1]]].append(qi_)
        # finalize chunks = pairs of quads (~8 tiles / 1 KB-wide Act ops);
        # per-chunk zbuf tiles keep the finalize pipeline from serializing
        # against one big buffer via tile-granular deps
        zchunks = [list(range(k, min(k + 2, nquads))) for k in range(0, nquads, 2)]
        zc_of_quad = {}
        for zc_, ql_ in enumerate(zchunks):
            off_ = 0
            for qi_ in ql_:
                zc_of_quad[qi_] = (zc_, off_)
                off_ += len(quads[qi_]) * P
        zwidth = [sum(len(quads[qi_]) for qi_ in ql_) * P for ql_ in zchunks]
        zbufs = [const.tile([P, zwidth[zc_]], BF16, name=f"zbuf{zc_}")
                 for zc_ in range(len(zchunks))]
        xtbuf = const.tile([P, shard], BF16)
        sacc = const.tile([P, nquads], F32)
        qacc = const.tile([P, nquads], F32)

        # first group's idx columns first (its gathers start ~1.5us earlier),
        # then the rest; xt deferred to the compute tail after the last gather
        # so it never delays gather descriptors
        nc.sync.dma_start(idx_s0[:], idx_d[:, 0:ic_split])
        nc.sync.dma_start(idx_s1[:], idx_d[:, ic_split:icols])
        nc.sync.dma_start(io_s[:], io_d[:])
        nc.sync.dma_start(dl_s[:], dl_d[:])
        nc.sync.dma_start(re_s[:], re_d[:])
        nc.sync.dma_start(wt_s[:], wt_d[:])
        nc.sync.dma_start(gb_s[:], gb_d[:])

        # dummy Sqrt: selects the act-function set covering Copy, Square,
        # Relu AND Sqrt ("sqrt_and_others"), so no mid-stream table reload
        # stalls the Act engine before the BN scalar chain
        epsb = stat.tile([P, 1], F32)
        nc.vector.memset(epsb[:], float(BN_EPS))
        warm = stat.tile([P, 1], F32)
        nc.scalar.activation(warm[:], epsb[:], AF.Sqrt)

        ev_view = xsbf[:, 0, :]   # [npad//2, D] stride 2*D
        od_view = xsbf[:, 1, :]

        ngroups = len(ginfo)
        group_msgs = {}
        for gi, gg in enumerate(ginfo):
            msgse = mpool.tile([P, gbmax, D], BF16, tag="msgse")
            msgso = mpool.tile([P, gbmax, D], BF16, tag="msgso")
            group_msgs[gi] = (msgse, msgso)
            for view, mt, calls in ((ev_view, msgse, gg["ev_calls"]),
                                    (od_view, msgso, gg["od_calls"])):
                for ic0, ln, blk0 in calls:
                    if ic0 < ic_split:
                        idx_ap = idx_s0[:, ic0 : ic0 + ln // 16]
                    else:
                        idx_ap = idx_s1[:, ic0 - ic_split : ic0 - ic_split + ln // 16]
                    nc.gpsimd.dma_gather(
                        mt[:, blk0 : blk0 + (-(-ln // P)), :],
                        view,
                        idx_ap,
                        ln,
                        ln,
                        elem_size=D,
                        elem_step=2 * D,
                    )

            for qi in quads_done_in_group[gi]:
                qs = quads[qi]
                w = len(qs) * P
                aggq = apool.tile([P, 4 * P], F32, tag="agg")
                for j, s in enumerate(qs):
                    prog = tile_prog[s]
                    msgs_q = group_msgs[group_of_slot[s]]
                    nk = len(prog)
                    for k, (q, mb, col) in enumerate(prog):
                        oh = ohpool.tile([P, P], BF16, tag="oh")
                        nc.vector.tensor_scalar(
                            oh[:], io_s[:],
                            dl_s[:, col : col + 1], re_s[:, col : col + 1],
                            AL.is_equal, AL.mult,
                        )
                        nc.tensor.matmul(
                            aggq[:, j * P : (j + 1) * P],
                            msgs_q[q][:, mb, :], oh[:],
                            start=(k == 0), stop=(k == nk - 1),
                        )

                h1q = hpool.tile([P, 4 * P], BF16, tag="h1")
                nc.scalar.activation(h1q[:, 0:w], aggq[:, 0:w], AF.Copy)

                zq = zpool.tile([P, 4 * P], F32, tag="zp")
                nc.tensor.matmul(zq[:, 0:w], wt_s[:, 0:D], h1q[:, 0:w],
                                 start=True, stop=False)
                nc.tensor.matmul(zq[:, 0:w], wt_s[:, D : 2 * D], h1q[:, 0:w],
                                 start=False, stop=True)

                # PSUM -> zbuf copy with Sum(z) accumulation; Square pass for
                # Sum(z^2). Both on the Act engine (PE/DVE stay free).
                zc, zoff = zc_of_quad[qi]
                nc.scalar.activation(
                    zbufs[zc][:, zoff : zoff + w], zq[:, 0:w],
                    AF.Copy, accum_out=sacc[:, qi : qi + 1],
                )
                sq = scpool.tile([P, 4 * P], BF16, tag="sq")
                nc.scalar.activation(
                    sq[:, 0:w], zq[:, 0:w], AF.Square,
                    accum_out=qacc[:, qi : qi + 1]
                )

        # residual stream issued on the Pool SWDGE queue: its descriptor gen
        # queues behind the final gather's, so the transfer lands in the
        # post-gather tail instead of ahead of the last gathers (the idle SP
        # engine would otherwise request the DMA unit early and push the
        # gather stream's end out by the full xt transfer time)
        nc.gpsimd.dma_start(xtbuf[:], xt_d[:])

        # ---- BatchNorm statistics (global over all cores) ----
        ssum = stat.tile([P, 2], F32)
        nc.vector.tensor_reduce(
            ssum[:, 0:1], sacc[:], axis=mybir.AxisListType.X, op=AL.add
        )
        nc.vector.tensor_reduce(
            ssum[:, 1:2], qacc[:], axis=mybir.AxisListType.X, op=AL.add
        )

        tot = stat.tile([P, 2], F32)
        if skip_collective:
            tot = ssum
        else:
            cin = dram.tile([P, 2], F32)
            cout = dram.tile([P, 2], F32)
            nc.gpsimd.dma_start(cin[:], ssum[:])
            nc.gpsimd.collective_compute(
                "AllReduce",
                AL.add,
                replica_groups=[list(range(ncores))],
                ins=[cin.opt()],
                outs=[cout.opt()],
            )
            nc.gpsimd.dma_start(tot[:], cout[:])

        # scale = gamma / sqrt(var+eps); shift = beta - mu*scale
        m2 = stat.tile([P, 2], F32)
        nc.vector.tensor_scalar(m2[:], tot[:], inv_n, None, AL.mult)
        mu = m2[:, 0:1]
        var = stat.tile([P, 1], F32)
        nc.vector.tensor_mul(var[:], m2[:, 0:1], m2[:, 0:1])
        nc.vector.tensor_sub(var[:], m2[:, 1:2], var[:])
        sd = stat.tile([P, 1], F32)
        nc.scalar.activation(sd[:], var[:], AF.Sqrt, bias=epsb[:])
        inv = stat.tile([P, 1], F32)
        nc.vector.reciprocal(inv[:], sd[:])
        scale = stat.tile([P, 1], F32)
        nc.vector.tensor_mul(scale[:], gb_s[:, 0:1], inv[:])
        shift = stat.tile([P, 1], F32)
        nc.vector.tensor_mul(shift[:], m2[:, 0:1], scale[:])
        nc.vector.tensor_sub(shift[:], gb_s[:, 1:2], shift[:])

        # ---- finalize: out = x + relu(z*scale + shift); xt was preloaded,
        # output written per chunk so the store DMA overlaps the compute ----
        # the first chunk runs on a pure-DVE path (relu(a)+x computed as
        # max(a+x, x); bf16 tensor_scalar hits the 4x DVE mode) while the Act
        # engine streams the remaining relu chunks in parallel -- the two
        # finalize streams overlap instead of serializing on Act. One chunk
        # on DVE balances the streams: DVE also runs every chunk's
        # residual-add, so giving it more relu chunks makes it the wall.
        for zc, ql in enumerate(zchunks):
            w = zwidth[zc]
            c0 = quads[ql[0]][0] * P
            sl = slice(c0, c0 + w)
            if zc < 1:
                t1 = fpool.tile([P, 8 * P], BF16, tag="fin")
                nc.vector.tensor_scalar(
                    t1[:, 0:w], zbufs[zc][:], scale[:], shift[:],
                    AL.mult, AL.add,
                )
                t2 = fpool.tile([P, 8 * P], BF16, tag="fin2")
                nc.vector.tensor_add(t2[:, 0:w], t1[:, 0:w], xtbuf[:, sl])
                obuf = fpool.tile([P, 8 * P], BF16, tag="obuf")
                nc.vector.tensor_tensor(
                    out=obuf[:, 0:w], in0=t2[:, 0:w], in1=xtbuf[:, sl],
                    op=AL.max,
                )
            else:
                tmp = fpool.tile([P, 8 * P], BF16, tag="fin")
                nc.scalar.activation(
                    tmp[:, 0:w], zbufs[zc][:], AF.Relu, bias=shift[:],
                    scale=scale[:]
                )
                obuf = fpool.tile([P, 8 * P], BF16, tag="obuf")
                nc.vector.tensor_add(obuf[:, 0:w], tmp[:, 0:w], xtbuf[:, sl])
            nc.sync.dma_start(out_d[:, sl], obuf[:, 0:w])

    nc.compile()
    return nc


# ---------------------------------------------------------------- entry point

def kernel(feature, W, b, gamma, beta, src, dst, _trace=False,
           _skip_collective=False):
    n = feature.shape[0]
    cfg, in_maps = host_prep(feature, W, gamma, beta, src, dst)
    nc = build_program(cfg, skip_collective=_skip_collective)

    def _run():
        # materialize results inside the attempt: device failures surface
        # lazily at array fetch, and must be covered by the retry
        r = run_bass_kernel_spmd(
            nc, in_maps, list(range(cfg["ncores"])), trace=_trace
        )
        outs = [np.asarray(r.results[c]["outt"]) for c in range(cfg["ncores"])]
        return r, outs

    try:
        res, outs = _run()
    except Exception:
        # retries: a previously-wedged device can fail the first attempts
        try:
            res, outs = _run()
        except Exception:
            res, outs = _run()
    full = np.empty((cfg["npad"], D), dtype=np.float32)
    for c in range(cfg["ncores"]):
        full[cfg["rows_of_core"][c]] = outs[c].T.astype(np.float32)
    out = full[:n]
    if _trace:
        return out, res
    return out

